# revision 8
# baseline (speedup 1.0000x reference)
"""Trainium2 Bass kernel for nn_AttModel (B=8, S=96, D=768, R=24, RSEQ=8, TAG=3).

Data-parallel over batch: core i handles sample i.
Per-core program (one sample):
  1. refine scan in score space: s_{t+1} = s_t + (scale*A@A.T) @ softmax(s_t),
     b_final.T = b0.T + A.T @ sum_t softmax(s_t)   (A fixed across steps)
  2. H projections, feature-major: HhT/HtT [2304 x 96], proj_b folded into HhT
  3. pairwise loop: V = relu(HtT + HhT[:, i]) per k-tile (bf16),
     out[72, i-block] = sum_k relW[k].T @ V[k]  accumulated in PSUM
Output per core: [72, 9216] with channel c = tag*24 + rel (rel_W pre-permuted
on host), reshaped on host to [3, 24, 96, 96].
"""
import sys

sys.path.insert(0, "/opt/trn_rl_repo")

import numpy as np

S, D, H3 = 96, 768, 2304
R, RSEQ, TAG, C = 24, 8, 3, 72
B = 8
KT = D // 128          # 6 k-tiles over D
MT = H3 // 128         # 18 m-tiles over 3D
IGRP = 4               # i's per output group
NG = S // IGRP         # 24 groups
NFREE = IGRP * S       # 384 moving free dim
DVE_K_N = 10           # k-tiles produced directly on DVE
ACT_K_N = 1            # k-tiles produced directly on ACT
# remaining MT - DVE_K_N - ACT_K_N k-tiles are PE-constructed + ACT relu-copy
SCALE = 1.0 / float(np.sqrt(np.float32(D)))


def build_nc(repeat: int = 1, skip_refine=False, skip_h=False, skip_main=False):
    import concourse.bass as bass
    from concourse import bacc, mybir
    import concourse.tile as tile
    from concourse.masks import make_identity

    f32 = mybir.dt.float32
    bf16 = mybir.dt.bfloat16
    AF = mybir.ActivationFunctionType
    ALU = mybir.AluOpType
    AX = mybir.AxisListType

    nc = bacc.Bacc()
    enc = nc.dram_tensor("enc", [S, D], f32, kind="ExternalInput")
    arel = nc.dram_tensor("arel", [RSEQ, D], f32, kind="ExternalInput")
    projW = nc.dram_tensor("projW", [2 * D, H3], f32, kind="ExternalInput")
    projb = nc.dram_tensor("projb", [H3], f32, kind="ExternalInput")
    relw = nc.dram_tensor("relw", [H3, C], f32, kind="ExternalInput")
    out = nc.dram_tensor("out", [C, S * S], f32, kind="ExternalOutput")

    dve_ks = list(range(DVE_K_N))
    act_ks = list(range(DVE_K_N, DVE_K_N + ACT_K_N))
    pe_ks = list(range(DVE_K_N + ACT_K_N, MT))

    with tile.TileContext(nc) as tc:
        with (
            tc.tile_pool(name="persist", bufs=1) as pp,
            tc.tile_pool(name="work", bufs=3) as wp,
            tc.tile_pool(name="vd", bufs=8) as vdp,
            tc.tile_pool(name="va", bufs=3) as vap,
            tc.tile_pool(name="vc", bufs=8) as vcp,
            tc.tile_pool(name="pst", bufs=2, space="PSUM") as pst,
            tc.tile_pool(name="pso", bufs=2, space="PSUM") as pso,
            tc.tile_pool(name="psx", bufs=3, space="PSUM") as psx,
            tc.tile_pool(name="pss", bufs=1, space="PSUM") as pss,
        ):

            def body(_it=None):
                # ---------- loads ----------
                ident = pp.tile([128, 128], f32, tag="ident")
                make_identity(nc, ident[:])
                identb = pp.tile([128, 128], bf16, tag="identb")
                make_identity(nc, identb[:])
                # sel3[s, i, j] = 1.0 where s == i else 0  (bf16)
                sel3 = pp.tile([96, 96, 96], bf16, tag="sel3")
                nc.gpsimd.memset(sel3[:], 0.0)
                nc.gpsimd.affine_select(
                    out=sel3[:], in_=sel3[:],
                    compare_op=mybir.AluOpType.not_equal, fill=1.0,
                    base=0, pattern=[[-1, 96], [0, 96]], channel_multiplier=1,
                )

                enc_nat = wp.tile([S, D], f32, tag="enc_nat")
                nc.scalar.dma_start(enc_nat[:], enc[:])
                a_nat = pp.tile([RSEQ, D], f32, tag="a_nat")
                nc.scalar.dma_start(a_nat[:], arel[:])
                pb_sb = pp.tile([128, MT], f32, tag="pb")
                nc.scalar.dma_start(
                    pb_sb[:], projb.rearrange("(t p) -> p t", p=128)
                )
                # rel_W -> f32 staging -> bf16 persistent
                rwr = []
                for k in range(MT):
                    stg = wp.tile([128, C], f32, tag="rw_stage")
                    nc.scalar.dma_start(stg[:], relw[k * 128:(k + 1) * 128, :])
                    t = pp.tile([128, C], bf16, tag=f"rwr{k}")
                    nc.vector.tensor_copy(t[:], stg[:])
                    rwr.append(t)
                # proj_W resident: 12 tiles [128, H3] (HWDGE fans out queues)
                pw = []
                for kt in range(2 * KT):
                    t = pp.tile([128, H3], f32, tag=f"pw{kt}")
                    nc.sync.dma_start(t[:], projW[kt * 128:(kt + 1) * 128, :])
                    pw.append(t)

                # ---------- transposes ----------
                # b.T tiles [128, 96] (feature-major enc)
                bT = []
                for k in range(KT):
                    ps = pst.tile([128, S], f32, tag="ps_t")
                    nc.tensor.transpose(
                        ps[:], enc_nat[:, k * 128:(k + 1) * 128], ident[:S, :S]
                    )
                    t = pp.tile([128, S], f32, tag=f"bT{k}")
                    nc.scalar.copy(t[:], ps[:])
                    bT.append(t)
                # A.T tiles [128, 8], raw + pre-scaled
                at_raw, at_scl = [], []
                for k in range(KT):
                    ps = pst.tile([128, RSEQ], f32, tag="ps_t")
                    nc.tensor.transpose(
                        ps[:], a_nat[:, k * 128:(k + 1) * 128],
                        ident[:RSEQ, :RSEQ],
                    )
                    tr = pp.tile([128, RSEQ], f32, tag=f"atr{k}")
                    nc.scalar.copy(tr[:], ps[:])
                    ts = pp.tile([128, RSEQ], f32, tag=f"ats{k}")
                    nc.scalar.mul(ts[:], ps[:], SCALE)
                    at_raw.append(tr)
                    at_scl.append(ts)

                # ---------- refine scan (score space) ----------
                # G' = scale * A @ A.T  [8, 8]
                gps = pst.tile([RSEQ, RSEQ], f32, tag="ps_t")
                for k in range(KT):
                    nc.tensor.matmul(
                        gps[:], at_scl[k][:], at_raw[k][:],
                        start=(k == 0), stop=(k == KT - 1),
                    )
                g_sb = pp.tile([RSEQ, RSEQ], f32, tag="g")
                nc.vector.tensor_copy(g_sb[:], gps[:])

                # s_psum = scale * A @ b0.T  [8, 96]; stays in PSUM all scan
                s_ps = pss.tile([RSEQ, S], f32, tag="s")
                for k in range(KT):
                    nc.tensor.matmul(
                        s_ps[:], at_scl[k][:], bT[k][:],
                        start=(k == 0), stop=False, skip_group_check=True,
                    )
                wsum = pp.tile([RSEQ, S], f32, tag="wsum")
                nc.vector.memset(wsum[:], 0.0)
                for t in range(0 if skip_refine else R):
                    negmax = wp.tile([RSEQ, 1], f32, tag="negmax")
                    nc.vector.reduce_max(
                        negmax[:], s_ps[:], axis=AX.X, negate=True
                    )
                    u = wp.tile([RSEQ, S], f32, tag="u")
                    rs = wp.tile([RSEQ, 1], f32, tag="rs")
                    nc.scalar.activation(
                        u[:], s_ps[:], AF.Exp, bias=negmax[:], scale=1.0,
                        accum_out=rs[:],
                    )
                    rinv = wp.tile([RSEQ, 1], f32, tag="rinv")
                    nc.vector.reciprocal(rinv[:], rs[:])
                    w = wp.tile([RSEQ, S], f32, tag="w")
                    nc.vector.tensor_scalar_mul(w[:], u[:], rinv[:])
                    nc.vector.tensor_tensor(
                        wsum[:], wsum[:], w[:], op=ALU.add
                    )
                    if t < R - 1:
                        nc.tensor.matmul(
                            s_ps[:], g_sb[:], w[:],
                            start=False, stop=(t == R - 2),
                            skip_group_check=True,
                        )
                # b_final.T = b0.T + A.T @ wsum
                for k in range(KT):
                    ps = pst.tile([128, S], f32, tag="ps_t")
                    nc.tensor.matmul(
                        ps[:], a_nat[:, k * 128:(k + 1) * 128], wsum[:],
                        start=True, stop=True,
                    )
                    nc.vector.tensor_tensor(
                        bT[k][:], bT[k][:], ps[:], op=ALU.add
                    )

                # ---------- H projections (feature-major, hh/ht interleaved) ----------
                hh, ht = [None] * MT, [None] * MT
                hh_nat = [None] * MT
                m_order = pe_ks + dve_ks + act_ks
                if skip_h:
                    for m in range(MT):
                        th = pp.tile([128, S], f32, tag=f"hh{m}")
                        nc.vector.memset(th[:], 0.01)
                        hh[m] = th
                        tt = pp.tile([128, S], bf16, tag=f"ht{m}")
                        nc.vector.memset(tt[:], 0.01)
                        ht[m] = tt
                        if m in pe_ks:
                            tn = pp.tile([96, 128], bf16, tag=f"hn{m}")
                            nc.vector.memset(tn[:], 0.01)
                            hh_nat[m] = tn
                for m in (m_order if not skip_h else []):
                    msl = slice(m * 128, (m + 1) * 128)
                    ps = pst.tile([128, S], f32, tag="ps_t")
                    for k in range(KT):
                        nc.tensor.matmul(
                            ps[:], pw[k][:, msl], bT[k][:],
                            start=(k == 0), stop=(k == KT - 1),
                        )
                    th = pp.tile([128, S], f32, tag=f"hh{m}")
                    # fold proj_b into HhT
                    nc.scalar.activation(
                        th[:], ps[:], AF.Identity,
                        bias=pb_sb[:, m:m + 1], scale=1.0,
                    )
                    hh[m] = th
                    ps2 = pst.tile([128, S], f32, tag="ps_t")
                    for k in range(KT):
                        nc.tensor.matmul(
                            ps2[:], pw[KT + k][:, msl], bT[k][:],
                            start=(k == 0), stop=(k == KT - 1),
                        )
                    tt = pp.tile([128, S], bf16, tag=f"ht{m}")
                    nc.vector.tensor_copy(tt[:], ps2[:])
                    ht[m] = tt
                    if m in pe_ks:
                        # hh_nat[m] = hh_feat[m].T [96, 128] bf16
                        psn = pst.tile([96, 128], f32, tag="ps_t")
                        nc.tensor.transpose(psn[:], hh[m][:], ident[:128, :128])
                        tn = pp.tile([96, 128], bf16, tag=f"hn{m}")
                        nc.vector.tensor_copy(tn[:], psn[:])
                        hh_nat[m] = tn

                # ---------- pairwise main loop ----------
                for ig in range(0 if skip_main else NG):
                    ops = pso.tile([C, NFREE], f32, tag="ops")
                    vtiles = {}
                    for k in pe_ks:
                        # X = Ht (bcast over i) + Hh rows (selector matmul)
                        x = psx.tile([128, NFREE], f32, tag="x")
                        hta = ht[k][:]
                        bcast = bass.AP(
                            tensor=hta.tensor, offset=hta.offset,
                            ap=[hta.ap[0], [0, IGRP], hta.ap[1]],
                        )
                        nc.tensor.matmul(
                            x[:], identb[:], bcast, start=True, stop=False,
                        )
                        nc.tensor.matmul(
                            x[:], hh_nat[k][:],
                            sel3[:, ig * IGRP:(ig + 1) * IGRP, :],
                            start=False, stop=True,
                        )
                        v = vcp.tile([128, NFREE], bf16, tag="vc")
                        nc.scalar.activation(v[:], x[:], AF.Relu, scale=1.0)
                        vtiles[k] = v
                    for k in dve_ks:
                        v = vdp.tile([128, NFREE], bf16, tag="vd")
                        for ii in range(IGRP):
                            i = ig * IGRP + ii
                            nc.vector.tensor_scalar(
                                v[:, ii * S:(ii + 1) * S], ht[k][:],
                                hh[k][:, i:i + 1], 0.0,
                                op0=ALU.add, op1=ALU.max,
                            )
                        vtiles[k] = v
                    for k in act_ks:
                        v = vap.tile([128, NFREE], bf16, tag="va")
                        for ii in range(IGRP):
                            i = ig * IGRP + ii
                            nc.scalar.activation(
                                v[:, ii * S:(ii + 1) * S], ht[k][:],
                                AF.Relu, bias=hh[k][:, i:i + 1], scale=1.0,
                            )
                        vtiles[k] = v
                    order = dve_ks + act_ks + pe_ks
                    for j, k in enumerate(order):
                        nc.tensor.matmul(
                            ops[:], rwr[k][:], vtiles[k][:],
                            start=(j == 0), stop=(j == MT - 1),
                        )
                    ostg = wp.tile([C, NFREE], f32, tag="ostg")
                    nc.scalar.copy(ostg[:], ops[:])
                    nc.sync.dma_start(
                        out[:, ig * NFREE:(ig + 1) * NFREE], ostg[:]
                    )

            if repeat == 1:
                body()
            else:
                with tc.For_i(0, repeat, 1) as it:
                    body(it)

    nc.finalize()
    return nc


_CACHED_NC = None


def _prep_in_maps(encoded_text, rel_types_encoded, proj_W, proj_b, rel_W):
    # permute rel_W columns: kernel channel c = tag*24 + rel reads original
    # column rel*3 + tag
    relw_perm = np.ascontiguousarray(
        rel_W.reshape(H3, R, TAG).transpose(0, 2, 1).reshape(H3, C)
    ).astype(np.float32)
    in_maps = []
    for i in range(B):
        in_maps.append({
            "enc": np.ascontiguousarray(encoded_text[i], dtype=np.float32),
            "arel": np.ascontiguousarray(
                rel_types_encoded[i], dtype=np.float32
            ),
            "projW": np.ascontiguousarray(proj_W, dtype=np.float32),
            "projb": np.ascontiguousarray(proj_b, dtype=np.float32),
            "relw": relw_perm,
        })
    return in_maps


def _assemble(results, rel_b):
    outs = []
    for i in range(B):
        o = results[i]["out"].reshape(TAG, R, S, S)
        outs.append(o)
    full = np.stack(outs, axis=0).astype(np.float32)  # [B, 3, 24, 96, 96]
    if np.any(rel_b):
        relb_perm = np.asarray(rel_b, dtype=np.float32).reshape(R, TAG).T
        full = full + relb_perm[None, :, :, None, None]
    return full


def kernel(encoded_text, rel_types_encoded, proj_W, proj_b, rel_W, rel_b):
    global _CACHED_NC
    from concourse.bass_utils import run_bass_kernel_spmd

    if _CACHED_NC is None:
        _CACHED_NC = build_nc(repeat=1)
    in_maps = _prep_in_maps(
        encoded_text, rel_types_encoded, proj_W, proj_b, rel_W
    )
    res = run_bass_kernel_spmd(_CACHED_NC, in_maps, list(range(B)))
    return _assemble(res.results, rel_b)
